# revision 8
# baseline (speedup 1.0000x reference)
import sys

sys.path.insert(0, "/opt/trn_rl_repo")

import numpy as np

import concourse.bass as bass
import concourse.bacc as bacc
import concourse.tile as tile
from concourse import mybir
from concourse.bass_utils import run_bass_kernel_spmd
from concourse.masks import make_identity

NCLS, BATCH, FEAT, HID = 10, 8192, 64, 128
NCORES = 8
BC = BATCH // NCORES  # batch per core
PT = 128  # partition tile
NT = BC // PT  # b-tiles per core
CH = 512  # matmul chunk (PSUM bank)
NCHUNK = BC // CH
F32 = mybir.dt.float32
BF16 = mybir.dt.bfloat16
AF = mybir.ActivationFunctionType
ALU = mybir.AluOpType
AX = mybir.AxisListType
NPAIR = NCLS * NCLS  # 100


def _ap(base, extra_off, dims):
    """Raw AP on the same tensor: partition dim kept, free dims replaced."""
    return bass.AP(
        tensor=base.tensor,
        offset=base.offset + extra_off,
        ap=[list(base.ap[0])] + [list(d) for d in dims],
    )


def build_program(m, c_total, d2s, has_b1=False):
    """m: number of positive-w2 columns (after permutation). c_total: logit
    bias (b2). d2s: final output bias.

    adjacency is kept centered: tv = tanh(L/2) = 2*sigmoid(L)-1, so
    c_ij = adj + I(+diag fix) = 0.5 + 0.5*tv + 1.5*delta_ij  (tv_ii := 0)
    Y_i = sum_j c_ij node_j = 0.5*(S + Yv_i + 3*node_i),  S = sum_j node_j
    the 0.5 is folded into G1 on the host.
    """
    KF = FEAT + 1 if has_b1 else FEAT  # extra ones-row only when b1 != 0
    nc = bacc.Bacc()
    node_d = nc.dram_tensor("node", [NCLS, BC, FEAT], F32, kind="ExternalInput")
    wpqab_d = nc.dram_tensor("wpqab", [KF, 130], F32, kind="ExternalInput")
    g1mat_d = nc.dram_tensor("g1mat", [FEAT, HID], F32, kind="ExternalInput")
    g1v_d = nc.dram_tensor("g1v", [HID, 1], F32, kind="ExternalInput")
    ecat_d = nc.dram_tensor("ecat", [HID, NCLS, HID], F32, kind="ExternalInput")
    cvec_d = nc.dram_tensor("cvec", [HID, 1], F32, kind="ExternalInput")
    d2v_d = nc.dram_tensor("d2v", [HID, 1], F32, kind="ExternalInput")
    out_d = nc.dram_tensor("out", [BC, 1], F32, kind="ExternalOutput")

    with tile.TileContext(nc) as tc:
        with (
            tc.tile_pool(name="singles", bufs=1) as singles,
            tc.tile_pool(name="nodebuf", bufs=1) as nodebuf,
            tc.tile_pool(name="nodet", bufs=3) as nodet_pool,
            tc.tile_pool(name="pqab", bufs=NT) as pqab_pool,
            tc.tile_pool(name="big", bufs=2) as big_pool,
            tc.tile_pool(name="small", bufs=4) as small_pool,
            tc.tile_pool(name="ybuf", bufs=2) as y_pool,
            tc.tile_pool(name="ytbuf", bufs=1) as yt_pool,
            tc.tile_pool(name="rbuf", bufs=3) as r_pool,
            tc.tile_pool(name="psT", bufs=2, space="PSUM") as psT_pool,
            tc.tile_pool(name="psPQ", bufs=2, space="PSUM") as psPQ_pool,
            tc.tile_pool(name="psZ", bufs=2, space="PSUM") as psZ_pool,
            tc.tile_pool(name="psV", bufs=2, space="PSUM") as psV_pool,
        ):
            # ---- constants / weights to SBUF ----
            identity = singles.tile([PT, PT], F32)
            make_identity(nc, identity[:])
            wpqab_sb = singles.tile([KF, 130], F32)
            nc.sync.dma_start(out=wpqab_sb[:], in_=wpqab_d[:])
            g1mat_sb = singles.tile([FEAT, HID], F32)
            nc.sync.dma_start(out=g1mat_sb[:], in_=g1mat_d[:])
            g1v_sb = singles.tile([HID, 1], F32)
            nc.sync.dma_start(out=g1v_sb[:], in_=g1v_d[:])
            ecat_f = singles.tile([HID, NCLS, HID], F32)
            nc.sync.dma_start(out=ecat_f[:], in_=ecat_d[:])
            ecat_sb = singles.tile([HID, NCLS, HID], BF16)
            nc.vector.tensor_copy(ecat_sb[:], ecat_f[:])
            cvec_sb = singles.tile([HID, 1], F32)
            nc.sync.dma_start(out=cvec_sb[:], in_=cvec_d[:])
            d2v_sb = singles.tile([HID, 1], F32)
            nc.sync.dma_start(out=d2v_sb[:], in_=d2v_d[:])

            # ---- node load: [128, NT, NCLS*64], class-major per tile ----
            nodeB = nodebuf.tile([PT, NT, NCLS * FEAT], F32)
            node_r = node_d[:].rearrange("i (t p) f -> p t i f", p=PT)
            for t in range(NT):
                nc.sync.dma_start(
                    out=nodeB[:, t, :].rearrange("p (i f) -> p i f", i=NCLS),
                    in_=node_r[:, t],
                )
            # f-major bf16 copy for the 2x aggregation multiply
            nodeFM = nodebuf.tile([PT, NT, FEAT * NCLS], BF16)

            # YT: [64, NCLS, NT, 128]
            YT = yt_pool.tile([FEAT, NCLS, NT, PT], F32)

            pqab_tiles = []
            for t in range(NT):
                nc.scalar.copy(
                    out=nodeFM[:, t, :].rearrange("p (f j) -> p f j", j=NCLS),
                    in_=nodeB[:, t, :]
                    .rearrange("p (j f) -> p j f", j=NCLS)
                    .transpose([0, 2, 1]),
                )
                # ---- nodeT: per-class transposes -> [65, NCLS, 128] ----
                nodeT = nodet_pool.tile([KF, NCLS, PT], F32)
                if has_b1:
                    nc.vector.memset(nodeT[FEAT : FEAT + 1, :, :], 1.0)
                for q in range(3):  # 4+4+2 classes per psum bank
                    lo = q * 4
                    hi = min(lo + 4, NCLS)
                    psT = psT_pool.tile([FEAT, 4 * PT], F32, tag="psT")
                    for i in range(lo, hi):
                        nc.tensor.transpose(
                            psT[:, (i - lo) * PT : (i - lo + 1) * PT],
                            nodeB[:, t, i * FEAT : (i + 1) * FEAT],
                            identity[:],
                        )
                    nc.scalar.copy(
                        out=nodeT[0:FEAT, lo:hi, :],
                        in_=psT[:, 0 : (hi - lo) * PT],
                    )

                # ---- P/Q/a/b: matmul lhsT=nodeT slice, rhs=wpqab ----
                pqab = pqab_pool.tile([PT, NCLS, 130], BF16)
                pqab_tiles.append(pqab)
                for q in range(4):  # 3+3+3+1 classes per psum bank
                    lo = q * 3
                    hi = min(lo + 3, NCLS)
                    psPQ = psPQ_pool.tile([PT, 390], F32, tag="psPQ")
                    for i in range(lo, hi):
                        nc.tensor.matmul(
                            psPQ[:, (i - lo) * 130 : (i - lo + 1) * 130],
                            nodeT[0:KF, i, :],
                            wpqab_sb[:],
                            start=True,
                            stop=True,
                        )
                    nc.scalar.copy(
                        out=pqab[:, lo:hi, :],
                        in_=psPQ[:, 0 : (hi - lo) * 130],
                    )

            for t in range(NT):
                pqab = pqab_tiles[t]
                # ---- pairwise X = P_i + Q_j  [128, 10, 10, 64] bf16 2x ----
                X = big_pool.tile([PT, NPAIR * FEAT], BF16, tag="bigX")
                in0 = (
                    pqab[:, :, 0:FEAT]
                    .unsqueeze(2)
                    .broadcast_to([PT, NCLS, NCLS, FEAT])
                )
                in1 = (
                    pqab[:, :, FEAT : 2 * FEAT]
                    .unsqueeze(1)
                    .broadcast_to([PT, NCLS, NCLS, FEAT])
                )
                nc.vector.tensor_add(
                    X[:].rearrange("p (i j k) -> p i j k", i=NCLS, j=NCLS), in0, in1
                )

                # ---- signed abs reduction over k ----
                Xv = X[:].rearrange("p (ij k) -> p ij k", k=FEAT)
                redP = small_pool.tile([PT, NPAIR], F32, tag="redP")
                redN = small_pool.tile([PT, NPAIR], F32, tag="redN")
                if m > 0:
                    nc.vector.tensor_reduce(
                        out=redP[:],
                        in_=Xv[:, :, 0:m],
                        axis=AX.X,
                        op=ALU.add,
                        apply_absolute_value=True,
                    )
                if m < FEAT:
                    nc.vector.tensor_reduce(
                        out=redN[:],
                        in_=Xv[:, :, m:FEAT],
                        axis=AX.X,
                        op=ALU.add,
                        apply_absolute_value=True,
                    )
                # rank1 = a_i + b_j
                rk = small_pool.tile([PT, NPAIR], F32, tag="rk")
                nc.vector.tensor_add(
                    rk[:].rearrange("p (i j) -> p i j", i=NCLS),
                    pqab[:, :, 128:129].broadcast_to([PT, NCLS, NCLS]),
                    pqab[:, :, 129:130]
                    .transpose([0, 2, 1])
                    .broadcast_to([PT, NCLS, NCLS]),
                )
                # t2 = rank1 + redP - redN
                t2 = small_pool.tile([PT, NPAIR], F32, tag="t2")
                if m == 0:
                    nc.vector.scalar_tensor_tensor(
                        out=t2[:], in0=redN[:], scalar=-1.0, in1=rk[:],
                        op0=ALU.mult, op1=ALU.add,
                    )
                elif m == FEAT:
                    nc.vector.tensor_add(t2[:], redP[:], rk[:])
                else:
                    t1 = small_pool.tile([PT, NPAIR], F32, tag="t1")
                    nc.vector.scalar_tensor_tensor(
                        out=t1[:], in0=redN[:], scalar=-1.0, in1=redP[:],
                        op0=ALU.mult, op1=ALU.add,
                    )
                    nc.vector.tensor_add(t2[:], t1[:], rk[:])

                # ---- tv = tanh(L/2) = tanh(0.25*t2 + C/2); diag := 0 ----
                tv = small_pool.tile([PT, NPAIR], BF16, tag="tv")
                nc.scalar.activation(
                    tv[:], t2[:], AF.Tanh, bias=float(c_total) * 0.5, scale=0.25
                )
                diag = _ap(tv[:], 0, [[NCLS + 1, NCLS]])
                nc.vector.memset(diag, 0.0)

                # ---- Yv_i = sum_j tv_ij * node_j   (i, f, j) bf16 2x ----
                prod = big_pool.tile([PT, NPAIR * FEAT], BF16, tag="bigX")
                nin = (
                    nodeFM[:, t, :]
                    .rearrange("p (f j) -> p f j", j=NCLS)
                    .unsqueeze(1)
                    .broadcast_to([PT, NCLS, FEAT, NCLS])
                )
                ain = (
                    tv[:]
                    .rearrange("p (i j) -> p i j", i=NCLS)
                    .unsqueeze(2)
                    .broadcast_to([PT, NCLS, FEAT, NCLS])
                )
                nc.vector.tensor_mul(
                    prod[:].rearrange("p (i f j) -> p i f j", i=NCLS, f=FEAT), nin, ain
                )
                Yv = y_pool.tile([PT, NCLS * FEAT], F32, tag="yv")
                nc.vector.tensor_reduce(
                    out=Yv[:],
                    in_=prod[:].rearrange("p (g j) -> p g j", j=NCLS),
                    axis=AX.X,
                    op=ALU.add,
                )
                # S = sum_j node_j (exact, from fp32 nodeB, strided inner)
                S = small_pool.tile([PT, FEAT], F32, tag="S")
                nc.vector.tensor_reduce(
                    out=S[:],
                    in_=nodeB[:, t, :]
                    .rearrange("p (j f) -> p j f", j=NCLS)
                    .transpose([0, 2, 1]),
                    axis=AX.X,
                    op=ALU.add,
                )
                # Y = Yv + S + 3*node   (the 0.5 is folded into G1)
                tsum = y_pool.tile([PT, NCLS * FEAT], F32, tag="tsum")
                nc.vector.tensor_add(
                    tsum[:].rearrange("p (i f) -> p i f", i=NCLS),
                    Yv[:].rearrange("p (i f) -> p i f", i=NCLS),
                    S[:].unsqueeze(1).broadcast_to([PT, NCLS, FEAT]),
                )
                Y = y_pool.tile([PT, NCLS * FEAT], F32, tag="y")
                nc.vector.scalar_tensor_tensor(
                    out=Y[:], in0=nodeB[:, t, :], scalar=3.0, in1=tsum[:],
                    op0=ALU.mult, op1=ALU.add,
                )

                # ---- Y transposes -> YT[:, i, t, :] ----
                for q in range(3):
                    lo = q * 4
                    hi = min(lo + 4, NCLS)
                    psY = psT_pool.tile([FEAT, 4 * PT], F32, tag="psT")
                    for i in range(lo, hi):
                        nc.tensor.transpose(
                            psY[:, (i - lo) * PT : (i - lo + 1) * PT],
                            Y[:, i * FEAT : (i + 1) * FEAT],
                            identity[:],
                        )
                    nc.scalar.copy(
                        out=YT[0:FEAT, lo:hi, t, :],
                        in_=psY[:, 0 : (hi - lo) * PT],
                    )

            # ---- decision head per 512-chunk ----
            for c in range(NCHUNK):
                psV = psV_pool.tile([HID, CH], F32)
                for i in range(NCLS):
                    psZ = psZ_pool.tile([HID, CH], F32, tag="z")
                    nc.tensor.matmul(
                        psZ[:],
                        g1mat_sb[:],
                        YT[0:FEAT, i, 4 * c : 4 * (c + 1), :],
                        start=True,
                        stop=True,
                    )
                    R = r_pool.tile([HID, CH], BF16, tag="r")
                    nc.scalar.activation(R[:], psZ[:], AF.Relu, bias=g1v_sb[:], scale=1.0)
                    nc.tensor.matmul(
                        psV[:],
                        ecat_sb[:, i, :],
                        R[:],
                        start=(i == 0),
                        stop=(i == NCLS - 1),
                    )
                R2 = r_pool.tile([HID, CH], F32, tag="r2")
                nc.scalar.activation(R2[:], psV[:], AF.Relu, bias=cvec_sb[:], scale=1.0)
                psO = psZ_pool.tile([1, CH], F32, tag="z")
                nc.tensor.matmul(psO[:], d2v_sb[:], R2[:], start=True, stop=True)
                osb = small_pool.tile([1, CH], F32, tag="osb")
                nc.scalar.activation(osb[:], psO[:], AF.Identity, bias=float(d2s), scale=1.0)
                nc.sync.dma_start(out=out_d[c * CH : (c + 1) * CH, :], in_=osb[:])

    nc.finalize()  # run bacc passes (reg alloc, 1-wait splitting) before serialization
    return nc


def _host_fold(inputs):
    W1 = np.asarray(inputs["W1"], np.float32)
    b1 = np.asarray(inputs["b1"], np.float32)
    W2 = np.asarray(inputs["W2"], np.float32)
    b2 = np.asarray(inputs["b2"], np.float32)
    G1 = np.asarray(inputs["G1"], np.float32)
    g1 = np.asarray(inputs["g1"], np.float32)
    G2 = np.asarray(inputs["G2"], np.float32)
    g2 = np.asarray(inputs["g2"], np.float32)
    D1 = np.asarray(inputs["D1"], np.float32)
    d1 = np.asarray(inputs["d1"], np.float32)
    D2 = np.asarray(inputs["D2"], np.float32)
    d2 = np.asarray(inputs["d2"], np.float32)

    s = W2[:, 0]
    pos = np.where(s > 0)[0]
    neg = np.where(s <= 0)[0]
    perm = np.concatenate([pos, neg])
    m = len(pos)
    sp = s[perm]

    W1a = W1[:FEAT][:, perm] * sp[None, :]
    W1b = W1[FEAT:][:, perm] * sp[None, :]
    acol = W1[:FEAT] @ s
    bcol = W1[FEAT:] @ s

    wpqab = np.zeros((FEAT + 1, 130), np.float32)
    wpqab[:FEAT, 0:FEAT] = W1a
    wpqab[:FEAT, FEAT : 2 * FEAT] = W1b
    wpqab[:FEAT, 128] = acol
    wpqab[:FEAT, 129] = bcol
    # ones-row: b1 fold goes to P-part and its sum to the a column
    wpqab[FEAT, 0:FEAT] = sp * b1[perm]
    wpqab[FEAT, 128] = float(np.dot(s, b1))

    c_total = float(b2[0])

    ecat = np.stack(
        [G2 @ D1[i * HID : (i + 1) * HID, :] for i in range(NCLS)], axis=1
    ).astype(np.float32)  # [128, 10, 128]
    cvec = (g2 @ D1.reshape(NCLS, HID, 128).sum(axis=0) + d1).astype(np.float32)

    has_b1 = bool(np.any(b1))
    if not has_b1:
        wpqab = wpqab[:FEAT]
    return {
        "m": m,
        "has_b1": has_b1,
        "c_total": c_total,
        "d2s": float(d2[0]),
        "wpqab": np.ascontiguousarray(wpqab),
        "g1mat": np.ascontiguousarray(0.5 * G1),  # 0.5 from centered adjacency
        "g1v": np.ascontiguousarray(g1[:, None]),
        "ecat": np.ascontiguousarray(ecat),
        "cvec": np.ascontiguousarray(cvec[:, None]),
        "d2v": np.ascontiguousarray(D2),
    }


def make_in_maps(inputs):
    fold = _host_fold(inputs)
    hs = np.asarray(inputs["hidden_state_list"], np.float32)
    weights = {
        k: fold[k] for k in ("wpqab", "g1mat", "g1v", "ecat", "cvec", "d2v")
    }
    in_maps = []
    for g in range(NCORES):
        im = dict(weights)
        im["node"] = np.ascontiguousarray(hs[:, g * BC : (g + 1) * BC, :])
        in_maps.append(im)
    return fold, in_maps


def kernel(**inputs):
    fold, in_maps = make_in_maps(inputs)
    nc = build_program(fold["m"], fold["c_total"], fold["d2s"], fold["has_b1"])
    res = run_bass_kernel_spmd(nc, in_maps, list(range(NCORES)))
    out = np.concatenate([res.results[g]["out"] for g in range(NCORES)], axis=0)
    return out.astype(np.float32)


# revision 12
# speedup vs baseline: 641.9465x; 641.9465x over previous
import sys

sys.path.insert(0, "/opt/trn_rl_repo")

import numpy as np

import concourse.bass as bass
import concourse.bacc as bacc
import concourse.tile as tile
from concourse import mybir
from concourse.bass_utils import run_bass_kernel_spmd
from concourse.masks import make_identity

NCLS, BATCH, FEAT, HID = 10, 8192, 64, 128
NCORES = 8
BC = BATCH // NCORES  # batch per core
PT = 128  # partition tile
NT = BC // PT  # b-tiles per core
CH = 256  # matmul chunk (half PSUM bank) — smaller head chunks overlap better
NCHUNK = BC // CH
TPC = NT // NCHUNK  # b-tiles per chunk
F32 = mybir.dt.float32
BF16 = mybir.dt.bfloat16
AF = mybir.ActivationFunctionType
ALU = mybir.AluOpType
AX = mybir.AxisListType
NPAIR = NCLS * NCLS  # 100


def _ap(base, extra_off, dims):
    """Raw AP on the same tensor: partition dim kept, free dims replaced."""
    return bass.AP(
        tensor=base.tensor,
        offset=base.offset + extra_off,
        ap=[list(base.ap[0])] + [list(d) for d in dims],
    )


def build_program(m, c_total, d2s, has_b1=False, repeat=1):
    """m: number of positive-w2 columns (after permutation). c_total: logit
    bias (b2). d2s: final output bias. repeat: re-run the whole compute
    `repeat` times via a hardware loop (for timing builds).

    adjacency is kept centered: tv = tanh(L/2) = 2*sigmoid(L)-1, so
    c_ij = adj + I(+diag fix) = 0.5 + 0.5*tv + 1.5*delta_ij  (tv_ii := 0)
    Y_i = sum_j c_ij node_j = 0.5*(S + Yv_i + 3*node_i),  S = sum_j node_j
    the 0.5 is folded into G1 on the host.
    """
    KF = FEAT + 1 if has_b1 else FEAT  # extra ones-row only when b1 != 0
    nc = bacc.Bacc()
    node_d = nc.dram_tensor("node", [NCLS, BC, FEAT], F32, kind="ExternalInput")
    wpqab_d = nc.dram_tensor("wpqab", [KF, 130], F32, kind="ExternalInput")
    g1mat_d = nc.dram_tensor("g1mat", [FEAT, HID], F32, kind="ExternalInput")
    g1v_d = nc.dram_tensor("g1v", [HID, 1], F32, kind="ExternalInput")
    ecat_d = nc.dram_tensor("ecat", [HID, NCLS, HID], F32, kind="ExternalInput")
    cvec_d = nc.dram_tensor("cvec", [HID, 1], F32, kind="ExternalInput")
    d2v_d = nc.dram_tensor("d2v", [HID, 1], F32, kind="ExternalInput")
    out_d = nc.dram_tensor("out", [BC, 1], F32, kind="ExternalOutput")

    with tile.TileContext(nc) as tc:
        with (
            tc.tile_pool(name="singles", bufs=1) as singles,
            tc.tile_pool(name="nodebuf", bufs=1) as nodebuf,
            tc.tile_pool(name="nodet", bufs=3) as nodet_pool,
            tc.tile_pool(name="pqab", bufs=4) as pqab_pool,
            tc.tile_pool(name="big", bufs=2) as big_pool,
            tc.tile_pool(name="small", bufs=4) as small_pool,
            tc.tile_pool(name="ybuf", bufs=2) as y_pool,
            tc.tile_pool(name="ytbuf", bufs=1) as yt_pool,
            tc.tile_pool(name="rbuf", bufs=3) as r_pool,
            tc.tile_pool(name="psT", bufs=2, space="PSUM") as psT_pool,
            tc.tile_pool(name="psPQ", bufs=2, space="PSUM") as psPQ_pool,
            tc.tile_pool(name="psZ", bufs=2, space="PSUM") as psZ_pool,
            tc.tile_pool(name="psV", bufs=2, space="PSUM") as psV_pool,
        ):
            # ---- constants / weights to SBUF ----
            identity = singles.tile([PT, PT], F32)
            make_identity(nc, identity[:])
            wpqab_sb = singles.tile([KF, 130], F32)
            nc.sync.dma_start(out=wpqab_sb[:], in_=wpqab_d[:])
            g1mat_sb = singles.tile([FEAT, HID], F32)
            nc.sync.dma_start(out=g1mat_sb[:], in_=g1mat_d[:])
            g1v_sb = singles.tile([HID, 1], F32)
            nc.sync.dma_start(out=g1v_sb[:], in_=g1v_d[:])
            ecat_f = singles.tile([HID, NCLS, HID], F32)
            nc.sync.dma_start(out=ecat_f[:], in_=ecat_d[:])
            ecat_sb = singles.tile([HID, NCLS, HID], BF16)
            nc.gpsimd.tensor_copy(ecat_sb[:], ecat_f[:])
            cvec_sb = singles.tile([HID, 1], F32)
            nc.sync.dma_start(out=cvec_sb[:], in_=cvec_d[:])
            d2v_sb = singles.tile([HID, 1], F32)
            nc.sync.dma_start(out=d2v_sb[:], in_=d2v_d[:])

            nodeB = nodebuf.tile([PT, NT, NCLS * FEAT], F32)
            nodeFM = nodebuf.tile([PT, NT, FEAT * NCLS], BF16)
            node_r = node_d[:].rearrange("i (t p) f -> p t i f", p=PT)
            # per-chunk YT so the head can start after half the tiles
            YT_list = []
            for ci in range(NCHUNK):
                ytc = yt_pool.tile([FEAT, NCLS, TPC, PT], F32, tag="yt%d" % ci)
                YT_list.append(ytc)

            def emit_body():
                pqab_tiles = {}

                def emit_prep(t):
                    # ---- node DMA for this tile ----
                    nc.sync.dma_start(
                        out=nodeB[:, t, :].rearrange("p (i f) -> p i f", i=NCLS),
                        in_=node_r[:, t],
                    )
                    # f-major bf16 copy for the 2x aggregation multiply
                    nc.gpsimd.tensor_copy(
                        nodeFM[:, t, :].rearrange("p (f j) -> p f j", j=NCLS),
                        nodeB[:, t, :]
                        .rearrange("p (j f) -> p j f", j=NCLS)
                        .transpose([0, 2, 1]),
                    )
                    # ---- nodeT: per-class transposes ----
                    nodeT = nodet_pool.tile([KF, NCLS, PT], F32)
                    if has_b1:
                        nc.vector.memset(nodeT[FEAT : FEAT + 1, :, :], 1.0)
                    for q in range(3):  # 4+4+2 classes per psum bank
                        lo = q * 4
                        hi = min(lo + 4, NCLS)
                        psT = psT_pool.tile([FEAT, 4 * PT], F32, tag="psT")
                        for i in range(lo, hi):
                            nc.tensor.transpose(
                                psT[:, (i - lo) * PT : (i - lo + 1) * PT],
                                nodeB[:, t, i * FEAT : (i + 1) * FEAT],
                                identity[:],
                            )
                        nc.scalar.copy(
                            out=nodeT[0:FEAT, lo:hi, :],
                            in_=psT[:, 0 : (hi - lo) * PT],
                        )
                    # ---- P/Q/a/b: matmul lhsT=nodeT slice, rhs=wpqab ----
                    pqab = pqab_pool.tile([PT, NCLS, 130], BF16)
                    pqab_tiles[t] = pqab
                    for q in range(4):  # 3+3+3+1 classes per psum bank
                        lo = q * 3
                        hi = min(lo + 3, NCLS)
                        psPQ = psPQ_pool.tile([PT, 390], F32, tag="psPQ")
                        for i in range(lo, hi):
                            nc.tensor.matmul(
                                psPQ[:, (i - lo) * 130 : (i - lo + 1) * 130],
                                nodeT[0:KF, i, :],
                                wpqab_sb[:],
                                start=True,
                                stop=True,
                            )
                        nc.scalar.copy(
                            out=pqab[:, lo:hi, :],
                            in_=psPQ[:, 0 : (hi - lo) * 130],
                        )

                def emit_chain(t):
                    pqab = pqab_tiles.pop(t)
                    # ---- pairwise X = P_i + Q_j, bf16 2x ----
                    X = big_pool.tile([PT, NPAIR * FEAT], BF16, tag="bigX")
                    in0 = (
                        pqab[:, :, 0:FEAT]
                        .unsqueeze(2)
                        .broadcast_to([PT, NCLS, NCLS, FEAT])
                    )
                    in1 = (
                        pqab[:, :, FEAT : 2 * FEAT]
                        .unsqueeze(1)
                        .broadcast_to([PT, NCLS, NCLS, FEAT])
                    )
                    nc.vector.tensor_add(
                        X[:].rearrange("p (i j k) -> p i j k", i=NCLS, j=NCLS),
                        in0,
                        in1,
                    )
                    # ---- signed abs reduction over k ----
                    Xv = X[:].rearrange("p (ij k) -> p ij k", k=FEAT)
                    redP = small_pool.tile([PT, NPAIR], F32, tag="redP")
                    redN = small_pool.tile([PT, NPAIR], F32, tag="redN")
                    if m > 0:
                        nc.vector.tensor_reduce(
                            out=redP[:],
                            in_=Xv[:, :, 0:m],
                            axis=AX.X,
                            op=ALU.add,
                            apply_absolute_value=True,
                        )
                    if m < FEAT:
                        nc.vector.tensor_reduce(
                            out=redN[:],
                            in_=Xv[:, :, m:FEAT],
                            axis=AX.X,
                            op=ALU.add,
                            apply_absolute_value=True,
                        )
                    # rank1 = a_i + b_j  (gpsimd)
                    rk = small_pool.tile([PT, NPAIR], F32, tag="rk")
                    nc.gpsimd.tensor_add(
                        rk[:].rearrange("p (i j) -> p i j", i=NCLS),
                        pqab[:, :, 128:129].broadcast_to([PT, NCLS, NCLS]),
                        pqab[:, :, 129:130]
                        .transpose([0, 2, 1])
                        .broadcast_to([PT, NCLS, NCLS]),
                    )
                    # t2 = rank1 + redP - redN  (gpsimd)
                    t2 = small_pool.tile([PT, NPAIR], F32, tag="t2")
                    if m == 0:
                        nc.gpsimd.tensor_sub(t2[:], rk[:], redN[:])
                    elif m == FEAT:
                        nc.gpsimd.tensor_add(t2[:], redP[:], rk[:])
                    else:
                        t1 = small_pool.tile([PT, NPAIR], F32, tag="t1")
                        nc.gpsimd.tensor_sub(t1[:], redP[:], redN[:])
                        nc.gpsimd.tensor_add(t2[:], t1[:], rk[:])

                    # ---- tv = tanh(0.25*t2 + C/2); diag := 0 ----
                    tv = small_pool.tile([PT, NPAIR], BF16, tag="tv")
                    nc.scalar.activation(
                        tv[:], t2[:], AF.Tanh, bias=float(c_total) * 0.5, scale=0.25
                    )
                    diag = _ap(tv[:], 0, [[NCLS + 1, NCLS]])
                    nc.vector.memset(diag, 0.0)

                    # ---- Yv_i = sum_j tv_ij * node_j  (i, f, j) bf16 2x ----
                    prod = big_pool.tile([PT, NPAIR * FEAT], BF16, tag="bigX")
                    nin = (
                        nodeFM[:, t, :]
                        .rearrange("p (f j) -> p f j", j=NCLS)
                        .unsqueeze(1)
                        .broadcast_to([PT, NCLS, FEAT, NCLS])
                    )
                    ain = (
                        tv[:]
                        .rearrange("p (i j) -> p i j", i=NCLS)
                        .unsqueeze(2)
                        .broadcast_to([PT, NCLS, FEAT, NCLS])
                    )
                    nc.vector.tensor_mul(
                        prod[:].rearrange("p (i f j) -> p i f j", i=NCLS, f=FEAT),
                        nin,
                        ain,
                    )
                    Yv = y_pool.tile([PT, NCLS * FEAT], F32, tag="yv")
                    nc.vector.tensor_reduce(
                        out=Yv[:],
                        in_=prod[:].rearrange("p (g j) -> p g j", j=NCLS),
                        axis=AX.X,
                        op=ALU.add,
                    )
                    # S = sum_j node_j (exact, fp32, strided inner)
                    S = small_pool.tile([PT, FEAT], F32, tag="S")
                    nc.vector.tensor_reduce(
                        out=S[:],
                        in_=nodeB[:, t, :]
                        .rearrange("p (j f) -> p j f", j=NCLS)
                        .transpose([0, 2, 1]),
                        axis=AX.X,
                        op=ALU.add,
                    )
                    # Y = Yv + S + 3*node   (the 0.5 lives in G1)
                    tsum = y_pool.tile([PT, NCLS * FEAT], F32, tag="tsum")
                    nc.gpsimd.tensor_add(
                        tsum[:].rearrange("p (i f) -> p i f", i=NCLS),
                        Yv[:].rearrange("p (i f) -> p i f", i=NCLS),
                        S[:].unsqueeze(1).broadcast_to([PT, NCLS, FEAT]),
                    )
                    Y = y_pool.tile([PT, NCLS * FEAT], F32, tag="y")
                    nc.vector.scalar_tensor_tensor(
                        out=Y[:], in0=nodeB[:, t, :], scalar=3.0, in1=tsum[:],
                        op0=ALU.mult, op1=ALU.add,
                    )
                    # ---- Y transposes -> YT[chunk][:, i, t%TPC, :] ----
                    YT = YT_list[t // TPC]
                    tt = t % TPC
                    for q in range(3):
                        lo = q * 4
                        hi = min(lo + 4, NCLS)
                        psY = psT_pool.tile([FEAT, 4 * PT], F32, tag="psT")
                        for i in range(lo, hi):
                            nc.tensor.transpose(
                                psY[:, (i - lo) * PT : (i - lo + 1) * PT],
                                Y[:, i * FEAT : (i + 1) * FEAT],
                                identity[:],
                            )
                        nc.scalar.copy(
                            out=YT[0:FEAT, lo:hi, tt, :],
                            in_=psY[:, 0 : (hi - lo) * PT],
                        )

                def emit_head(c):
                    psV = psV_pool.tile([HID, CH], F32)
                    for i in range(NCLS):
                        psZ = psZ_pool.tile([HID, CH], F32, tag="z")
                        nc.tensor.matmul(
                            psZ[:],
                            g1mat_sb[:],
                            YT_list[c][0:FEAT, i, :, :],
                            start=True,
                            stop=True,
                        )
                        R = r_pool.tile([HID, CH], BF16, tag="r")
                        nc.scalar.activation(
                            R[:], psZ[:], AF.Relu, bias=g1v_sb[:], scale=1.0
                        )
                        nc.tensor.matmul(
                            psV[:],
                            ecat_sb[:, i, :],
                            R[:],
                            start=(i == 0),
                            stop=(i == NCLS - 1),
                        )
                    R2 = r_pool.tile([HID, CH], F32, tag="r2")
                    nc.scalar.activation(
                        R2[:], psV[:], AF.Relu, bias=cvec_sb[:], scale=1.0
                    )
                    psO = psZ_pool.tile([1, CH], F32, tag="z")
                    nc.tensor.matmul(psO[:], d2v_sb[:], R2[:], start=True, stop=True)
                    osb = small_pool.tile([1, CH], F32, tag="osb")
                    nc.scalar.activation(
                        osb[:], psO[:], AF.Identity, bias=float(d2s), scale=1.0
                    )
                    nc.sync.dma_start(out=out_d[c * CH : (c + 1) * CH, :], in_=osb[:])

                # software-pipelined emission: prep runs 2 tiles ahead;
                # each head chunk is emitted as soon as its tiles are done
                for t in range(NT + 2):
                    if t < NT:
                        emit_prep(t)
                    if t >= 2:
                        tc_ = t - 2
                        emit_chain(tc_)
                        if (tc_ + 1) % TPC == 0:
                            emit_head((tc_ + 1) // TPC - 1)

            if repeat > 1:
                with tc.For_i(0, repeat, 1):
                    emit_body()
            else:
                emit_body()

    nc.finalize()  # run bacc passes (reg alloc, 1-wait splitting) before serialize
    return nc


def _host_fold(inputs):
    W1 = np.asarray(inputs["W1"], np.float32)
    b1 = np.asarray(inputs["b1"], np.float32)
    W2 = np.asarray(inputs["W2"], np.float32)
    b2 = np.asarray(inputs["b2"], np.float32)
    G1 = np.asarray(inputs["G1"], np.float32)
    g1 = np.asarray(inputs["g1"], np.float32)
    G2 = np.asarray(inputs["G2"], np.float32)
    g2 = np.asarray(inputs["g2"], np.float32)
    D1 = np.asarray(inputs["D1"], np.float32)
    d1 = np.asarray(inputs["d1"], np.float32)
    D2 = np.asarray(inputs["D2"], np.float32)
    d2 = np.asarray(inputs["d2"], np.float32)

    s = W2[:, 0]
    pos = np.where(s > 0)[0]
    neg = np.where(s <= 0)[0]
    perm = np.concatenate([pos, neg])
    m = len(pos)
    sp = s[perm]

    W1a = W1[:FEAT][:, perm] * sp[None, :]
    W1b = W1[FEAT:][:, perm] * sp[None, :]
    acol = W1[:FEAT] @ s
    bcol = W1[FEAT:] @ s

    wpqab = np.zeros((FEAT + 1, 130), np.float32)
    wpqab[:FEAT, 0:FEAT] = W1a
    wpqab[:FEAT, FEAT : 2 * FEAT] = W1b
    wpqab[:FEAT, 128] = acol
    wpqab[:FEAT, 129] = bcol
    # ones-row: b1 fold goes to P-part and its sum to the a column
    wpqab[FEAT, 0:FEAT] = sp * b1[perm]
    wpqab[FEAT, 128] = float(np.dot(s, b1))

    c_total = float(b2[0])

    ecat = np.stack(
        [G2 @ D1[i * HID : (i + 1) * HID, :] for i in range(NCLS)], axis=1
    ).astype(np.float32)  # [128, 10, 128]
    cvec = (g2 @ D1.reshape(NCLS, HID, 128).sum(axis=0) + d1).astype(np.float32)

    has_b1 = bool(np.any(b1))
    if not has_b1:
        wpqab = wpqab[:FEAT]
    return {
        "m": m,
        "has_b1": has_b1,
        "c_total": c_total,
        "d2s": float(d2[0]),
        "wpqab": np.ascontiguousarray(wpqab),
        "g1mat": np.ascontiguousarray(0.5 * G1),  # 0.5 from centered adjacency
        "g1v": np.ascontiguousarray(g1[:, None]),
        "ecat": np.ascontiguousarray(ecat),
        "cvec": np.ascontiguousarray(cvec[:, None]),
        "d2v": np.ascontiguousarray(D2),
    }


def make_in_maps(inputs):
    fold = _host_fold(inputs)
    hs = np.asarray(inputs["hidden_state_list"], np.float32)
    weights = {
        k: fold[k] for k in ("wpqab", "g1mat", "g1v", "ecat", "cvec", "d2v")
    }
    in_maps = []
    for g in range(NCORES):
        im = dict(weights)
        im["node"] = np.ascontiguousarray(hs[:, g * BC : (g + 1) * BC, :])
        in_maps.append(im)
    return fold, in_maps


def kernel(**inputs):
    fold, in_maps = make_in_maps(inputs)
    nc = build_program(fold["m"], fold["c_total"], fold["d2s"], fold["has_b1"])
    res = run_bass_kernel_spmd(nc, in_maps, list(range(NCORES)))
    out = np.concatenate([res.results[g]["out"] for g in range(NCORES)], axis=0)
    return out.astype(np.float32)


# revision 14
# speedup vs baseline: 2022.6609x; 3.1508x over previous
import sys

sys.path.insert(0, "/opt/trn_rl_repo")

import os

import numpy as np

import concourse.bass as bass
import concourse.bacc as bacc
import concourse.tile as tile
from concourse import mybir
from concourse.bass_utils import run_bass_kernel_spmd
from concourse.masks import make_identity

NCLS, BATCH, FEAT, HID = 10, 8192, 64, 128
NCORES = 8
BC = BATCH // NCORES  # batch per core
PT = 128  # partition tile
NT = BC // PT  # b-tiles per core
CH = 256  # matmul chunk (half PSUM bank) — smaller head chunks overlap better
NCHUNK = BC // CH
TPC = NT // NCHUNK  # b-tiles per chunk
F32 = mybir.dt.float32
BF16 = mybir.dt.bfloat16
AF = mybir.ActivationFunctionType
ALU = mybir.AluOpType
AX = mybir.AxisListType
NPAIR = NCLS * NCLS  # 100
USE_GPS = os.environ.get("KERNEL_USE_GPSIMD", "") == "1"


def _ap(base, extra_off, dims):
    """Raw AP on the same tensor: partition dim kept, free dims replaced."""
    return bass.AP(
        tensor=base.tensor,
        offset=base.offset + extra_off,
        ap=[list(base.ap[0])] + [list(d) for d in dims],
    )


def build_program(m, c_total, d2s, has_b1=False, repeat=1):
    """m: number of positive-w2 columns (after permutation). c_total: logit
    bias (b2). d2s: final output bias. repeat: re-run the whole compute
    `repeat` times via a hardware loop (for timing builds).

    adjacency is kept centered: tv = tanh(L/2) = 2*sigmoid(L)-1, so
    c_ij = adj + I(+diag fix) = 0.5 + 0.5*tv + 1.5*delta_ij  (tv_ii := 0)
    Y_i = sum_j c_ij node_j = 0.5*(S + Yv_i + 3*node_i),  S = sum_j node_j
    the 0.5 is folded into G1 on the host.
    """
    KF = FEAT + 1 if has_b1 else FEAT  # extra ones-row only when b1 != 0
    nc = bacc.Bacc()
    node_d = nc.dram_tensor("node", [NCLS, BC, FEAT], F32, kind="ExternalInput")
    wpqab_d = nc.dram_tensor("wpqab", [KF, 130], F32, kind="ExternalInput")
    g1mat_d = nc.dram_tensor("g1mat", [FEAT, HID], F32, kind="ExternalInput")
    g1v_d = nc.dram_tensor("g1v", [HID, 1], F32, kind="ExternalInput")
    ecat_d = nc.dram_tensor("ecat", [HID, NCLS, HID], F32, kind="ExternalInput")
    cvec_d = nc.dram_tensor("cvec", [HID, 1], F32, kind="ExternalInput")
    d2v_d = nc.dram_tensor("d2v", [HID, 1], F32, kind="ExternalInput")
    out_d = nc.dram_tensor("out", [BC, 1], F32, kind="ExternalOutput")

    with tile.TileContext(nc) as tc:
        with (
            tc.tile_pool(name="singles", bufs=1) as singles,
            tc.tile_pool(name="nodebuf", bufs=1) as nodebuf,
            tc.tile_pool(name="nodet", bufs=3) as nodet_pool,
            tc.tile_pool(name="pqab", bufs=4) as pqab_pool,
            tc.tile_pool(name="big", bufs=2) as big_pool,
            tc.tile_pool(name="small", bufs=4) as small_pool,
            tc.tile_pool(name="ybuf", bufs=2) as y_pool,
            tc.tile_pool(name="ytbuf", bufs=1) as yt_pool,
            tc.tile_pool(name="rbuf", bufs=3) as r_pool,
            tc.tile_pool(name="psT", bufs=2, space="PSUM") as psT_pool,
            tc.tile_pool(name="psPQ", bufs=2, space="PSUM") as psPQ_pool,
            tc.tile_pool(name="psZ", bufs=2, space="PSUM") as psZ_pool,
            tc.tile_pool(name="psV", bufs=2, space="PSUM") as psV_pool,
        ):
            # ---- constants / weights to SBUF ----
            identity = singles.tile([PT, PT], F32)
            make_identity(nc, identity[:])
            wpqab_sb = singles.tile([KF, 130], F32)
            nc.sync.dma_start(out=wpqab_sb[:], in_=wpqab_d[:])
            g1mat_sb = singles.tile([FEAT, HID], F32)
            nc.sync.dma_start(out=g1mat_sb[:], in_=g1mat_d[:])
            g1v_sb = singles.tile([HID, 1], F32)
            nc.sync.dma_start(out=g1v_sb[:], in_=g1v_d[:])
            ecat_f = singles.tile([HID, NCLS, HID], F32)
            nc.sync.dma_start(out=ecat_f[:], in_=ecat_d[:])
            ecat_sb = singles.tile([HID, NCLS, HID], BF16)
            (nc.gpsimd if USE_GPS else nc.vector).tensor_copy(ecat_sb[:], ecat_f[:])
            cvec_sb = singles.tile([HID, 1], F32)
            nc.sync.dma_start(out=cvec_sb[:], in_=cvec_d[:])
            d2v_sb = singles.tile([HID, 1], F32)
            nc.sync.dma_start(out=d2v_sb[:], in_=d2v_d[:])

            nodeB = nodebuf.tile([PT, NT, NCLS * FEAT], F32)
            nodeFM = nodebuf.tile([PT, NT, FEAT * NCLS], BF16)
            node_r = node_d[:].rearrange("i (t p) f -> p t i f", p=PT)
            # per-chunk YT so the head can start after half the tiles
            YT_list = []
            for ci in range(NCHUNK):
                ytc = yt_pool.tile([FEAT, NCLS, TPC, PT], F32, tag="yt%d" % ci)
                YT_list.append(ytc)

            def emit_body():
                pqab_tiles = {}

                def emit_prep(t):
                    # ---- node DMA for this tile ----
                    nc.sync.dma_start(
                        out=nodeB[:, t, :].rearrange("p (i f) -> p i f", i=NCLS),
                        in_=node_r[:, t],
                    )
                    # f-major bf16 copy for the 2x aggregation multiply
                    (nc.gpsimd if USE_GPS else nc.vector).tensor_copy(
                        nodeFM[:, t, :].rearrange("p (f j) -> p f j", j=NCLS),
                        nodeB[:, t, :]
                        .rearrange("p (j f) -> p j f", j=NCLS)
                        .transpose([0, 2, 1]),
                    )
                    # ---- nodeT: per-class transposes ----
                    nodeT = nodet_pool.tile([KF, NCLS, PT], F32)
                    if has_b1:
                        nc.vector.memset(nodeT[FEAT : FEAT + 1, :, :], 1.0)
                    for q in range(3):  # 4+4+2 classes per psum bank
                        lo = q * 4
                        hi = min(lo + 4, NCLS)
                        psT = psT_pool.tile([FEAT, 4 * PT], F32, tag="psT")
                        for i in range(lo, hi):
                            nc.tensor.transpose(
                                psT[:, (i - lo) * PT : (i - lo + 1) * PT],
                                nodeB[:, t, i * FEAT : (i + 1) * FEAT],
                                identity[:],
                            )
                        nc.scalar.copy(
                            out=nodeT[0:FEAT, lo:hi, :],
                            in_=psT[:, 0 : (hi - lo) * PT],
                        )
                    # ---- P/Q/a/b: matmul lhsT=nodeT slice, rhs=wpqab ----
                    pqab = pqab_pool.tile([PT, NCLS, 130], BF16)
                    pqab_tiles[t] = pqab
                    for q in range(4):  # 3+3+3+1 classes per psum bank
                        lo = q * 3
                        hi = min(lo + 3, NCLS)
                        psPQ = psPQ_pool.tile([PT, 390], F32, tag="psPQ")
                        for i in range(lo, hi):
                            nc.tensor.matmul(
                                psPQ[:, (i - lo) * 130 : (i - lo + 1) * 130],
                                nodeT[0:KF, i, :],
                                wpqab_sb[:],
                                start=True,
                                stop=True,
                            )
                        nc.scalar.copy(
                            out=pqab[:, lo:hi, :],
                            in_=psPQ[:, 0 : (hi - lo) * 130],
                        )

                def emit_chain(t):
                    pqab = pqab_tiles.pop(t)
                    # ---- pairwise X = P_i + Q_j, bf16 2x ----
                    X = big_pool.tile([PT, NPAIR * FEAT], BF16, tag="bigX")
                    in0 = (
                        pqab[:, :, 0:FEAT]
                        .unsqueeze(2)
                        .broadcast_to([PT, NCLS, NCLS, FEAT])
                    )
                    in1 = (
                        pqab[:, :, FEAT : 2 * FEAT]
                        .unsqueeze(1)
                        .broadcast_to([PT, NCLS, NCLS, FEAT])
                    )
                    nc.vector.tensor_add(
                        X[:].rearrange("p (i j k) -> p i j k", i=NCLS, j=NCLS),
                        in0,
                        in1,
                    )
                    # ---- signed abs reduction over k ----
                    Xv = X[:].rearrange("p (ij k) -> p ij k", k=FEAT)
                    redP = small_pool.tile([PT, NPAIR], F32, tag="redP")
                    redN = small_pool.tile([PT, NPAIR], F32, tag="redN")
                    if m > 0:
                        nc.vector.tensor_reduce(
                            out=redP[:],
                            in_=Xv[:, :, 0:m],
                            axis=AX.X,
                            op=ALU.add,
                            apply_absolute_value=True,
                        )
                    if m < FEAT:
                        nc.vector.tensor_reduce(
                            out=redN[:],
                            in_=Xv[:, :, m:FEAT],
                            axis=AX.X,
                            op=ALU.add,
                            apply_absolute_value=True,
                        )
                    # rank1 = a_i + b_j  (gpsimd)
                    rk = small_pool.tile([PT, NPAIR], F32, tag="rk")
                    (nc.gpsimd if USE_GPS else nc.vector).tensor_add(
                        rk[:].rearrange("p (i j) -> p i j", i=NCLS),
                        pqab[:, :, 128:129].broadcast_to([PT, NCLS, NCLS]),
                        pqab[:, :, 129:130]
                        .transpose([0, 2, 1])
                        .broadcast_to([PT, NCLS, NCLS]),
                    )
                    # t2 = rank1 + redP - redN  (gpsimd)
                    t2 = small_pool.tile([PT, NPAIR], F32, tag="t2")
                    if m == 0:
                        (nc.gpsimd if USE_GPS else nc.vector).tensor_sub(t2[:], rk[:], redN[:])
                    elif m == FEAT:
                        (nc.gpsimd if USE_GPS else nc.vector).tensor_add(t2[:], redP[:], rk[:])
                    else:
                        t1 = small_pool.tile([PT, NPAIR], F32, tag="t1")
                        (nc.gpsimd if USE_GPS else nc.vector).tensor_sub(t1[:], redP[:], redN[:])
                        (nc.gpsimd if USE_GPS else nc.vector).tensor_add(t2[:], t1[:], rk[:])

                    # ---- tv = tanh(0.25*t2 + C/2); diag := 0 ----
                    tv = small_pool.tile([PT, NPAIR], BF16, tag="tv")
                    nc.scalar.activation(
                        tv[:], t2[:], AF.Tanh, bias=float(c_total) * 0.5, scale=0.25
                    )
                    diag = _ap(tv[:], 0, [[NCLS + 1, NCLS]])
                    nc.vector.memset(diag, 0.0)

                    # ---- Yv_i = sum_j tv_ij * node_j  (i, f, j) bf16 2x ----
                    prod = big_pool.tile([PT, NPAIR * FEAT], BF16, tag="bigX")
                    nin = (
                        nodeFM[:, t, :]
                        .rearrange("p (f j) -> p f j", j=NCLS)
                        .unsqueeze(1)
                        .broadcast_to([PT, NCLS, FEAT, NCLS])
                    )
                    ain = (
                        tv[:]
                        .rearrange("p (i j) -> p i j", i=NCLS)
                        .unsqueeze(2)
                        .broadcast_to([PT, NCLS, FEAT, NCLS])
                    )
                    nc.vector.tensor_mul(
                        prod[:].rearrange("p (i f j) -> p i f j", i=NCLS, f=FEAT),
                        nin,
                        ain,
                    )
                    Yv = y_pool.tile([PT, NCLS * FEAT], F32, tag="yv")
                    nc.vector.tensor_reduce(
                        out=Yv[:],
                        in_=prod[:].rearrange("p (g j) -> p g j", j=NCLS),
                        axis=AX.X,
                        op=ALU.add,
                    )
                    # S = sum_j node_j (exact, fp32, strided inner)
                    S = small_pool.tile([PT, FEAT], F32, tag="S")
                    nc.vector.tensor_reduce(
                        out=S[:],
                        in_=nodeB[:, t, :]
                        .rearrange("p (j f) -> p j f", j=NCLS)
                        .transpose([0, 2, 1]),
                        axis=AX.X,
                        op=ALU.add,
                    )
                    # Y = Yv + S + 3*node   (the 0.5 lives in G1)
                    tsum = y_pool.tile([PT, NCLS * FEAT], F32, tag="tsum")
                    (nc.gpsimd if USE_GPS else nc.vector).tensor_add(
                        tsum[:].rearrange("p (i f) -> p i f", i=NCLS),
                        Yv[:].rearrange("p (i f) -> p i f", i=NCLS),
                        S[:].unsqueeze(1).broadcast_to([PT, NCLS, FEAT]),
                    )
                    Y = y_pool.tile([PT, NCLS * FEAT], F32, tag="y")
                    nc.vector.scalar_tensor_tensor(
                        out=Y[:], in0=nodeB[:, t, :], scalar=3.0, in1=tsum[:],
                        op0=ALU.mult, op1=ALU.add,
                    )
                    # ---- Y transposes -> YT[chunk][:, i, t%TPC, :] ----
                    YT = YT_list[t // TPC]
                    tt = t % TPC
                    for q in range(3):
                        lo = q * 4
                        hi = min(lo + 4, NCLS)
                        psY = psT_pool.tile([FEAT, 4 * PT], F32, tag="psT")
                        for i in range(lo, hi):
                            nc.tensor.transpose(
                                psY[:, (i - lo) * PT : (i - lo + 1) * PT],
                                Y[:, i * FEAT : (i + 1) * FEAT],
                                identity[:],
                            )
                        nc.scalar.copy(
                            out=YT[0:FEAT, lo:hi, tt, :],
                            in_=psY[:, 0 : (hi - lo) * PT],
                        )

                def emit_head(c):
                    psV = psV_pool.tile([HID, CH], F32)
                    for i in range(NCLS):
                        psZ = psZ_pool.tile([HID, CH], F32, tag="z")
                        nc.tensor.matmul(
                            psZ[:],
                            g1mat_sb[:],
                            YT_list[c][0:FEAT, i, :, :],
                            start=True,
                            stop=True,
                        )
                        R = r_pool.tile([HID, CH], BF16, tag="r")
                        nc.scalar.activation(
                            R[:], psZ[:], AF.Relu, bias=g1v_sb[:], scale=1.0
                        )
                        nc.tensor.matmul(
                            psV[:],
                            ecat_sb[:, i, :],
                            R[:],
                            start=(i == 0),
                            stop=(i == NCLS - 1),
                        )
                    R2 = r_pool.tile([HID, CH], F32, tag="r2")
                    nc.scalar.activation(
                        R2[:], psV[:], AF.Relu, bias=cvec_sb[:], scale=1.0
                    )
                    psO = psZ_pool.tile([1, CH], F32, tag="z")
                    nc.tensor.matmul(psO[:], d2v_sb[:], R2[:], start=True, stop=True)
                    osb = small_pool.tile([1, CH], F32, tag="osb")
                    nc.scalar.activation(
                        osb[:], psO[:], AF.Identity, bias=float(d2s), scale=1.0
                    )
                    nc.sync.dma_start(out=out_d[c * CH : (c + 1) * CH, :], in_=osb[:])

                # software-pipelined emission: prep runs 2 tiles ahead;
                # each head chunk is emitted as soon as its tiles are done
                for t in range(NT + 2):
                    if t < NT:
                        emit_prep(t)
                    if t >= 2:
                        tc_ = t - 2
                        emit_chain(tc_)
                        if (tc_ + 1) % TPC == 0:
                            emit_head((tc_ + 1) // TPC - 1)

            if repeat > 1:
                with tc.For_i(0, repeat, 1):
                    emit_body()
            else:
                emit_body()

    nc.finalize()  # run bacc passes (reg alloc, 1-wait splitting) before serialize
    return nc


def _host_fold(inputs):
    W1 = np.asarray(inputs["W1"], np.float32)
    b1 = np.asarray(inputs["b1"], np.float32)
    W2 = np.asarray(inputs["W2"], np.float32)
    b2 = np.asarray(inputs["b2"], np.float32)
    G1 = np.asarray(inputs["G1"], np.float32)
    g1 = np.asarray(inputs["g1"], np.float32)
    G2 = np.asarray(inputs["G2"], np.float32)
    g2 = np.asarray(inputs["g2"], np.float32)
    D1 = np.asarray(inputs["D1"], np.float32)
    d1 = np.asarray(inputs["d1"], np.float32)
    D2 = np.asarray(inputs["D2"], np.float32)
    d2 = np.asarray(inputs["d2"], np.float32)

    s = W2[:, 0]
    pos = np.where(s > 0)[0]
    neg = np.where(s <= 0)[0]
    perm = np.concatenate([pos, neg])
    m = len(pos)
    sp = s[perm]

    W1a = W1[:FEAT][:, perm] * sp[None, :]
    W1b = W1[FEAT:][:, perm] * sp[None, :]
    acol = W1[:FEAT] @ s
    bcol = W1[FEAT:] @ s

    wpqab = np.zeros((FEAT + 1, 130), np.float32)
    wpqab[:FEAT, 0:FEAT] = W1a
    wpqab[:FEAT, FEAT : 2 * FEAT] = W1b
    wpqab[:FEAT, 128] = acol
    wpqab[:FEAT, 129] = bcol
    # ones-row: b1 fold goes to P-part and its sum to the a column
    wpqab[FEAT, 0:FEAT] = sp * b1[perm]
    wpqab[FEAT, 128] = float(np.dot(s, b1))

    c_total = float(b2[0])

    ecat = np.stack(
        [G2 @ D1[i * HID : (i + 1) * HID, :] for i in range(NCLS)], axis=1
    ).astype(np.float32)  # [128, 10, 128]
    cvec = (g2 @ D1.reshape(NCLS, HID, 128).sum(axis=0) + d1).astype(np.float32)

    has_b1 = bool(np.any(b1))
    if not has_b1:
        wpqab = wpqab[:FEAT]
    return {
        "m": m,
        "has_b1": has_b1,
        "c_total": c_total,
        "d2s": float(d2[0]),
        "wpqab": np.ascontiguousarray(wpqab),
        "g1mat": np.ascontiguousarray(0.5 * G1),  # 0.5 from centered adjacency
        "g1v": np.ascontiguousarray(g1[:, None]),
        "ecat": np.ascontiguousarray(ecat),
        "cvec": np.ascontiguousarray(cvec[:, None]),
        "d2v": np.ascontiguousarray(D2),
    }


def make_in_maps(inputs):
    fold = _host_fold(inputs)
    hs = np.asarray(inputs["hidden_state_list"], np.float32)
    weights = {
        k: fold[k] for k in ("wpqab", "g1mat", "g1v", "ecat", "cvec", "d2v")
    }
    in_maps = []
    for g in range(NCORES):
        im = dict(weights)
        im["node"] = np.ascontiguousarray(hs[:, g * BC : (g + 1) * BC, :])
        in_maps.append(im)
    return fold, in_maps


def kernel(**inputs):
    fold, in_maps = make_in_maps(inputs)
    nc = build_program(fold["m"], fold["c_total"], fold["d2s"], fold["has_b1"])
    res = run_bass_kernel_spmd(nc, in_maps, list(range(NCORES)))
    out = np.concatenate([res.results[g]["out"] for g in range(NCORES)], axis=0)
    return out.astype(np.float32)
